# revision 36
# baseline (speedup 1.0000x reference)
"""Trainium2 Bass kernel for nn_MultiHeadAttention_52527450030314.

Ragged-head multi-head attention:
  qkv = x @ w_qkv + b_qkv ; per-head ragged slices ; softmax(q k^T / sqrt(d)) v
  out = concat(heads) @ w_out + b_out
Returns (output[B,T,D], loss_scalar) matching the reference pytree.

Sharding: 8 cores; core c handles batch c//2, query-half c%2. Each core's
per-core input x is the full 2048-token batch, rolled so its 1024 query
tokens are rows 0..1023 (softmax over keys is permutation invariant).

On-chip layout is feature-major ("transposed"): X^T, Q^T, K^T are [d, tokens]
with d on partitions. PE matmul operands must start at partition 0 (quadrant
rule), so Q^T/K^T are built per head, each in its own partition-0-based tile.
Scores are computed transposed S^T[tk, tq]; softmax denominators come from an
all-ones column appended per head to V (the PV matmul then emits sum_tk P^T
as an extra output row); the denominator row is DMA'd to partition 0,
reciprocal'd, broadcast via a rank-1 matmul, and multiplied in. No
max-subtraction is needed: scores have unit-scale statistics (|s| < ~7), so
exp() is well-conditioned in fp32.
"""

import sys

sys.path.insert(0, "/opt/trn_rl_repo")

import math
from contextlib import ExitStack

import numpy as np

import concourse.bass as bass
import concourse.mybir as mybir
import concourse.tile as tile
from concourse import bacc
from concourse.masks import make_identity
from concourse.bass_utils import run_bass_kernel_spmd

D_MODEL = 512
NUM_HEADS = 6
MIN_DIM = 8
B = 4
T = 2048
TQ = 1024  # query tokens per core
P = 128
N_CORES = 8

F32 = mybir.dt.float32
F32R = mybir.dt.float32r
AF = mybir.ActivationFunctionType
ALU = mybir.AluOpType

# dtype used for the PE matmuls (fp32 data; float32r = fast relaxed path)
MM_DT = F32R


def _compute_integer_dims(head_dims_float):
    hdf = np.asarray(head_dims_float)
    hdi = np.floor(hdf).astype(np.int64)
    diff = int(D_MODEL - hdi.sum())
    res = hdf - np.floor(hdf)
    if diff > 0:
        idx = np.argsort(-res)[:diff]
        hdi[idx] += 1
    elif diff < 0:
        idx = np.argsort(res)[:(-diff)]
        hdi[idx] -= 1
    return hdi


def _host_head_dims_and_loss(head_weight_logits):
    """Head dims + aux loss, matching the f32 reference computation."""
    x = np.asarray(head_weight_logits, dtype=np.float32)
    try:  # prefer jax-on-cpu to match the reference bit-for-bit
        import jax

        cpu = jax.devices("cpu")[0]
        with jax.default_device(cpu):
            import jax.numpy as jnp

            xa = jnp.asarray(x)
            ratios = jax.nn.softmax(xa, axis=0)
            hdf_j = MIN_DIM + ratios * (D_MODEL - MIN_DIM * NUM_HEADS)
            hdf = np.asarray(hdf_j)
            hdi = _compute_integer_dims(hdf)
            hdi_f = jnp.asarray(hdi, dtype=hdf_j.dtype)
            float_loss = (hdf_j.sum() - D_MODEL) ** 2 + jnp.mean(
                jax.nn.relu(MIN_DIM - hdf_j)
            )
            int_loss = jnp.mean((hdf_j - hdi_f) ** 2)
            loss = np.asarray(float_loss + 0.5 * int_loss, dtype=np.float32)
        return [int(d) for d in hdi], loss
    except Exception:
        m = x.max()
        e = np.exp(x - m)
        ratios = e / e.sum()
        hdf = np.float32(MIN_DIM) + ratios * np.float32(D_MODEL - MIN_DIM * NUM_HEADS)
        hdi = _compute_integer_dims(hdf)
        hdi_f = hdi.astype(np.float32)
        float_loss = (hdf.sum() - np.float32(D_MODEL)) ** 2 + np.mean(
            np.maximum(np.float32(MIN_DIM) - hdf, np.float32(0.0))
        )
        int_loss = np.mean((hdf - hdi_f) ** 2)
        loss = np.float32(float_loss + np.float32(0.5) * int_loss)
        return [int(d) for d in hdi], loss


def build_nc(dims):
    dims = [int(d) for d in dims]
    assert sum(dims) == D_MODEL and len(dims) == NUM_HEADS
    offs = [0]
    for d in dims:
        offs.append(offs[-1] + d)
    aug = [offs[i] + i for i in range(NUM_HEADS)]  # aug col start per head
    AUGW = D_MODEL + NUM_HEADS
    NT = T // P      # 16 key-token tiles
    NTP = T // 256   # 8 key-tile pairs
    NQC = TQ // 512  # 2 query chunks

    # per-head row chunks of d_i (for d_i > 128); and of d_i + 1 for PV out
    def row_chunks(n):
        ch = []
        cs = 0
        while cs < n:
            ch.append((cs, min(P, n - cs)))
            cs += P
        return ch

    qk_chunks = [row_chunks(d) for d in dims]          # K^T/Q^T row chunks
    pv_chunks = [row_chunks(d + 1) for d in dims]      # PV out rows (+sum row)

    nc = bacc.Bacc()
    x_d = nc.dram_tensor("x", [T, D_MODEL], F32, kind="ExternalInput")
    wqkv_d = nc.dram_tensor("w_qkv", [D_MODEL, 3 * D_MODEL], F32, kind="ExternalInput")
    bqkv_d = nc.dram_tensor("b_qkv", [3 * D_MODEL], F32, kind="ExternalInput")
    wout_d = nc.dram_tensor("w_out", [D_MODEL, D_MODEL], F32, kind="ExternalInput")
    bout_d = nc.dram_tensor("b_out", [D_MODEL], F32, kind="ExternalInput")
    y_d = nc.dram_tensor("y", [TQ, D_MODEL], F32, kind="ExternalOutput")

    with ExitStack() as ctx:
        tc = ctx.enter_context(tile.TileContext(nc))
        sb = ctx.enter_context(tc.tile_pool(name="sb", bufs=1))
        ps = ctx.enter_context(tc.tile_pool(name="ps", bufs=1, space="PSUM"))

        # ---- constants ----
        ident = sb.tile([P, P], F32, tag="ident")
        make_identity(nc, ident)
        ones = sb.tile([P, P], F32, tag="ones")
        nc.gpsimd.memset(ones[:], 1.0)

        # ---- per-head q/k biases, partition-major, base 0 ----
        # bqh col (i)          : b_q rows of head i chunk c at col 2*i+c... use map
        bias_cols = {}  # (which, i, c) -> col   which: 0=q, 1=k
        ncol = 0
        for which in range(2):
            for i in range(NUM_HEADS):
                for c in range(len(qk_chunks[i])):
                    bias_cols[(which, i, c)] = ncol
                    ncol += 1
        bqh = sb.tile([P, ncol], F32, tag="bqh")
        for which in range(2):
            for i in range(NUM_HEADS):
                for ci, (cs, m) in enumerate(qk_chunks[i]):
                    col = bias_cols[(which, i, ci)]
                    src = bqkv_d[which * D_MODEL + offs[i] + cs:
                                 which * D_MODEL + offs[i] + cs + m]
                    nc.sync.dma_start(
                        bqh[0:m, col:col + 1], src.rearrange("(a b) -> a b", b=1)
                    )

        # ---- b_v / b_out broadcast to all partitions via rank-1 matmul ----
        bvsrc = sb.tile([1, D_MODEL], F32, tag="row", bufs=2, name="bvsrc")
        nc.sync.dma_start(bvsrc[:], bqkv_d[1024:1536].rearrange("(a b) -> a b", a=1))
        bosrc = sb.tile([1, D_MODEL], F32, tag="row", bufs=2, name="bosrc")
        nc.sync.dma_start(bosrc[:], bout_d[:].rearrange("(a b) -> a b", a=1))
        bvb = sb.tile([P, D_MODEL], F32, tag="bvb")
        bob = sb.tile([P, D_MODEL], F32, tag="bob")
        for src, dst in ((bvsrc, bvb), (bosrc, bob)):
            bc_ps = ps.tile([P, 512], F32, tag="pv", bufs=2, name=f"bcb_{dst.tensor.name}")
            nc.tensor.matmul(
                bc_ps[:, 0:D_MODEL],
                lhsT=ones[0:1, 0:P],
                rhs=src[0:1, :],
                start=True, stop=True,
            )
            nc.vector.tensor_copy(dst[:], bc_ps[:, 0:D_MODEL])

        # ---- weights ----
        # stage DMA loads, then round-copy: walrus requires f32r matmul
        # operands to be produced (rounded) by a compute engine, not DMA
        wq = []
        for j in range(4):
            w = sb.tile([P, 3 * D_MODEL], MM_DT, tag="wqkv", bufs=4, name=f"wqkv{j}")
            for cchunk in range(3):
                stg = sb.tile([P, D_MODEL], F32, tag="xn", bufs=3, name=f"wstg{j}_{cchunk}")
                nc.sync.dma_start(
                    stg[:], wqkv_d[j * P:(j + 1) * P, cchunk * 512:(cchunk + 1) * 512]
                )
                nc.vector.tensor_copy(
                    w[:, cchunk * 512:(cchunk + 1) * 512], stg[:]
                )
            wq.append(w)
        # w_out rows per (head, chunk), each base-0 in its own tile
        wos = {}
        n_wo = sum(len(qk_chunks[i]) for i in range(NUM_HEADS))
        for i in range(NUM_HEADS):
            for ci, (cs, m) in enumerate(qk_chunks[i]):
                w = sb.tile([P, D_MODEL], MM_DT, tag="wos", bufs=n_wo, name=f"wos{i}_{ci}")
                stg = sb.tile([P, D_MODEL], F32, tag="xn", bufs=3, name=f"wostg{i}_{ci}")
                nc.sync.dma_start(
                    stg[0:m, :], wout_d[offs[i] + cs: offs[i] + cs + m, :]
                )
                nc.vector.tensor_copy(w[0:m, :], stg[0:m, :])
                wos[(i, ci)] = w

        # ---- phase A: load X, transpose to X^T ----
        xt = [sb.tile([P, T], MM_DT, tag="xt", bufs=4, name=f"xt{j}") for j in range(4)]
        for t in range(NT):
            xn = sb.tile([P, D_MODEL], F32, tag="xn", bufs=3, name=f"xn{t}")
            nc.sync.dma_start(xn[:], x_d[t * P:(t + 1) * P, :])
            for j in range(4):
                tp = ps.tile([P, P], F32, tag="s", bufs=2, name=f"tp{t}_{j}")
                nc.tensor.transpose(tp[:], xn[:, j * P:(j + 1) * P], ident[:])
                nc.vector.tensor_copy(
                    xt[j][:, t * P:(t + 1) * P], tp[:]
                )

        # ---- phase B: V projection into augmented layout ----
        # V natural [2048, 518]: head i at cols aug[i]..aug[i]+d_i, ones col after
        vau = [sb.tile([P, AUGW], MM_DT, tag="vau", bufs=NT, name=f"vau{t}")
               for t in range(NT)]
        for t in range(NT):
            for i in range(NUM_HEADS):
                nc.vector.tensor_copy(
                    vau[t][:, aug[i] + dims[i]: aug[i] + dims[i] + 1],
                    ones[:, 0:1],
                )
        for t in range(NT):
            acc = ps.tile([P, 512], F32, tag="pa", bufs=2, name=f"v{t}")
            for j in range(4):
                nc.tensor.matmul(
                    acc[:],
                    lhsT=xt[j][:, t * P:(t + 1) * P],
                    rhs=wq[j][:, 1024:1536],
                    start=(j == 0),
                    stop=(j == 3),
                )
            for i in range(NUM_HEADS):
                nc.vector.tensor_tensor(
                    vau[t][:, aug[i]: aug[i] + dims[i]],
                    acc[:, offs[i]: offs[i] + dims[i]],
                    bvb[:, offs[i]: offs[i] + dims[i]],
                    ALU.add,
                )

        # ---- phase C: attention, head-major ----
        ao = {}
        for i in range(NUM_HEADS):
            for ci in range(len(pv_chunks[i])):
                ao[(i, ci)] = sb.tile([P, TQ], MM_DT, tag=f"ao{i}_{ci}", name=f"ao{i}_{ci}")

        mc = max(len(qk_chunks[i]) for i in range(NUM_HEADS))

        def emit_production(i):
            # K^T_i [d, 2048] and Q^T_i [d, 1024], per 128-row chunk, base 0
            kth, qth = [], []
            for ci, (cs, m) in enumerate(qk_chunks[i]):
                kc = sb.tile([P, T], MM_DT, tag="ktmp", bufs=2 * mc, name=f"kt{i}_{ci}")
                qc_t = sb.tile([P, TQ], MM_DT, tag="qtmp", bufs=2 * mc, name=f"qt{i}_{ci}")
                for ntile, woff, dst, bwhich in (
                    (T // 512, D_MODEL, kc, 1),
                    (TQ // 512, 0, qc_t, 0),
                ):
                    wcol = woff + offs[i] + cs
                    bcol = bias_cols[(bwhich, i, ci)]
                    for n in range(ntile):
                        acc = ps.tile([P, 512], F32, tag="pa", bufs=2,
                                      name=f"qk{i}_{ci}_{woff}_{n}")
                        for j in range(4):
                            nc.tensor.matmul(
                                acc[0:m, :],
                                lhsT=wq[j][:, wcol: wcol + m],
                                rhs=xt[j][:, n * 512:(n + 1) * 512],
                                start=(j == 0),
                                stop=(j == 3),
                            )
                        nc.vector.tensor_scalar_add(
                            dst[0:m, n * 512:(n + 1) * 512],
                            acc[0:m, :],
                            bqh[0:m, bcol:bcol + 1],
                        )
                kth.append(kc)
                qth.append(qc_t)
            return kth, qth

        def emit_attention(i, kth, qth):
            d = dims[i]
            scale = 1.0 / math.sqrt(d)
            nqk = len(qk_chunks[i])
            for qc in range(NQC):
                pvs = [
                    ps.tile([P, 512], F32, tag="pv", bufs=2, name=f"pv{i}_{qc}_{ci}")
                    for ci in range(len(pv_chunks[i]))
                ]
                for tkp in range(NTP):
                    sc = ps.tile([P, 1024], F32, tag="s", bufs=2, name=f"sc{i}_{qc}_{tkp}")
                    for half in range(2):
                        tk = tkp * 2 + half
                        for ci, (cs, m) in enumerate(qk_chunks[i]):
                            nc.tensor.matmul(
                                sc[:, half * 512:(half + 1) * 512],
                                lhsT=kth[ci][0:m, tk * P:(tk + 1) * P],
                                rhs=qth[ci][0:m, qc * 512:(qc + 1) * 512],
                                start=(ci == 0),
                                stop=(ci == nqk - 1),
                            )
                    e = sb.tile([P, 1024], MM_DT, tag="e", bufs=3, name=f"e{i}_{qc}_{tkp}")
                    nc.scalar.activation(e[:], sc[:], AF.Exp, scale=scale)
                    for half in range(2):
                        tk = tkp * 2 + half
                        for ci, (cs, m) in enumerate(pv_chunks[i]):
                            nc.tensor.matmul(
                                pvs[ci][0:m, :],
                                lhsT=vau[tk][:, aug[i] + cs: aug[i] + cs + m],
                                rhs=e[:, half * 512:(half + 1) * 512],
                                start=(tkp == 0 and half == 0),
                                stop=(tkp == NTP - 1 and half == 1),
                            )
                # normalize: divide PV rows by the denominator row
                cl = d // P        # chunk holding the sum row
                sr = d - cl * P    # its local row
                pvsb = []
                for ci, (cs, m) in enumerate(pv_chunks[i]):
                    t_sb = sb.tile([P, 512], F32, tag="pvsb", bufs=3,
                                   name=f"pvsb{i}_{qc}_{ci}")
                    nc.vector.tensor_copy(t_sb[0:m, :], pvs[ci][0:m, :])
                    pvsb.append(t_sb)
                rcs = sb.tile([1, 512], F32, tag="row", bufs=2, name=f"rcs{i}_{qc}")
                nc.sync.dma_start(rcs[0:1, :], pvsb[cl][sr:sr + 1, :])
                # broadcast the raw sum row to 128 partitions, then take the
                # reciprocal at full width (single-partition DVE ops are slow)
                bc_ps = ps.tile([P, 512], F32, tag="pv", bufs=2, name=f"bcp{i}_{qc}")
                nc.tensor.matmul(
                    bc_ps[:],
                    lhsT=ones[0:1, 0:P],
                    rhs=rcs[0:1, :],
                    start=True, stop=True,
                )
                bcs = sb.tile([P, 512], F32, tag="bcy", bufs=2, name=f"bcs{i}_{qc}")
                nc.vector.reciprocal(bcs[:], bc_ps[:])
                for ci, (cs, m) in enumerate(pv_chunks[i]):
                    mr = min(P, max(0, d - cs))  # real rows (excl. sum row)
                    if mr == 0:
                        continue
                    nc.vector.tensor_tensor(
                        ao[(i, ci)][0:mr, qc * 512:(qc + 1) * 512],
                        pvsb[ci][0:mr, :],
                        bcs[0:mr, :],
                        ALU.mult,
                    )

        # pipeline: emit head i+1's projections before head i's attention so
        # the PE has production work while ACT paces the exp stream
        prod = emit_production(0)
        for i in range(NUM_HEADS):
            nxt = emit_production(i + 1) if i + 1 < NUM_HEADS else None
            emit_attention(i, prod[0], prod[1])
            prod = nxt

        # ---- phase D: output projection ----
        pieces = []
        for i in range(NUM_HEADS):
            for ci, (cs, m) in enumerate(qk_chunks[i]):
                pieces.append((i, ci, m))
        for tt in range(TQ // P):
            yp = ps.tile([P, 512], F32, tag="s", bufs=2, name=f"yp{tt}")
            for pi, (i, ci, m) in enumerate(pieces):
                nc.tensor.matmul(
                    yp[:],
                    lhsT=ao[(i, ci)][0:m, tt * P:(tt + 1) * P],
                    rhs=wos[(i, ci)][0:m, :],
                    start=(pi == 0),
                    stop=(pi == len(pieces) - 1),
                )
            ysb = sb.tile([P, D_MODEL], F32, tag="bcy", bufs=2, name=f"ysb{tt}")
            nc.vector.tensor_tensor(ysb[:], yp[:], bob[:], ALU.add)
            nc.sync.dma_start(y_d[tt * P:(tt + 1) * P, :], ysb[:])

    nc.finalize()
    return nc


# test-harness knobs (the graded path leaves these at defaults)
TRACE = False
LAST_RESULTS = None

_NC_CACHE = {}


def _get_nc(dims):
    key = tuple(dims)
    if key not in _NC_CACHE:
        _NC_CACHE[key] = build_nc(dims)
    return _NC_CACHE[key]


def kernel(query, head_weight_logits, w_qkv, b_qkv, w_out, b_out):
    query = np.ascontiguousarray(np.asarray(query, dtype=np.float32))
    w_qkv = np.ascontiguousarray(np.asarray(w_qkv, dtype=np.float32))
    b_qkv = np.ascontiguousarray(np.asarray(b_qkv, dtype=np.float32))
    w_out = np.ascontiguousarray(np.asarray(w_out, dtype=np.float32))
    b_out = np.ascontiguousarray(np.asarray(b_out, dtype=np.float32))

    dims, loss = _host_head_dims_and_loss(head_weight_logits)
    nc = _get_nc(dims)

    in_maps = []
    for c in range(N_CORES):
        b = c // 2
        h = c % 2
        if h == 0:
            xc = query[b]
        else:
            xc = np.concatenate([query[b, TQ:], query[b, :TQ]], axis=0)
        in_maps.append({
            "x": np.ascontiguousarray(xc),
            "w_qkv": w_qkv,
            "b_qkv": b_qkv,
            "w_out": w_out,
            "b_out": b_out,
        })

    kwargs = {}
    if TRACE:
        kwargs = dict(trace=True)
    res = run_bass_kernel_spmd(nc, in_maps, core_ids=list(range(N_CORES)), **kwargs)
    global LAST_RESULTS
    LAST_RESULTS = res
    out = np.empty((B, T, D_MODEL), dtype=np.float32)
    for c in range(N_CORES):
        b = c // 2
        h = c % 2
        out[b, h * TQ:(h + 1) * TQ] = res.results[c]["y"]
    return out, loss


# revision 38
# speedup vs baseline: 1.3729x; 1.3729x over previous
"""Trainium2 Bass kernel for nn_MultiHeadAttention_52527450030314.

Ragged-head multi-head attention:
  qkv = x @ w_qkv + b_qkv ; per-head ragged slices ; softmax(q k^T / sqrt(d)) v
  out = concat(heads) @ w_out + b_out
Returns (output[B,T,D], loss_scalar) matching the reference pytree.

Sharding: 8 cores; core c handles batch c//2, query-half c%2. Each core's
per-core input x is the full 2048-token batch, rolled so its 1024 query
tokens are rows 0..1023 (softmax over keys is permutation invariant).

On-chip layout is feature-major ("transposed"): X^T, Q^T, K^T are [d, tokens]
with d on partitions. PE matmul operands must start at partition 0 (quadrant
rule), so Q^T/K^T are built per head, each in its own partition-0-based tile.
Scores are computed transposed S^T[tk, tq]; softmax denominators come from an
all-ones column appended per head to V (the PV matmul then emits sum_tk P^T
as an extra output row); the denominator row is DMA'd to partition 0,
reciprocal'd, broadcast via a rank-1 matmul, and multiplied in. No
max-subtraction is needed: scores have unit-scale statistics (|s| < ~7), so
exp() is well-conditioned in fp32.
"""

import sys

sys.path.insert(0, "/opt/trn_rl_repo")

import math
from contextlib import ExitStack

import numpy as np

import concourse.bass as bass
import concourse.mybir as mybir
import concourse.tile as tile
from concourse import bacc
from concourse.masks import make_identity
from concourse.bass_utils import run_bass_kernel_spmd

D_MODEL = 512
NUM_HEADS = 6
MIN_DIM = 8
B = 4
T = 2048
TQ = 1024  # query tokens per core
P = 128
N_CORES = 8

F32 = mybir.dt.float32
F32R = mybir.dt.float32r
AF = mybir.ActivationFunctionType
ALU = mybir.AluOpType

# dtype used for the PE matmuls (fp32 data; float32r = fast relaxed path)
MM_DT = F32R


def _compute_integer_dims(head_dims_float):
    hdf = np.asarray(head_dims_float)
    hdi = np.floor(hdf).astype(np.int64)
    diff = int(D_MODEL - hdi.sum())
    res = hdf - np.floor(hdf)
    if diff > 0:
        idx = np.argsort(-res)[:diff]
        hdi[idx] += 1
    elif diff < 0:
        idx = np.argsort(res)[:(-diff)]
        hdi[idx] -= 1
    return hdi


def _host_head_dims_and_loss(head_weight_logits):
    """Head dims + aux loss, matching the f32 reference computation."""
    x = np.asarray(head_weight_logits, dtype=np.float32)
    try:  # prefer jax-on-cpu to match the reference bit-for-bit
        import jax

        cpu = jax.devices("cpu")[0]
        with jax.default_device(cpu):
            import jax.numpy as jnp

            xa = jnp.asarray(x)
            ratios = jax.nn.softmax(xa, axis=0)
            hdf_j = MIN_DIM + ratios * (D_MODEL - MIN_DIM * NUM_HEADS)
            hdf = np.asarray(hdf_j)
            hdi = _compute_integer_dims(hdf)
            hdi_f = jnp.asarray(hdi, dtype=hdf_j.dtype)
            float_loss = (hdf_j.sum() - D_MODEL) ** 2 + jnp.mean(
                jax.nn.relu(MIN_DIM - hdf_j)
            )
            int_loss = jnp.mean((hdf_j - hdi_f) ** 2)
            loss = np.asarray(float_loss + 0.5 * int_loss, dtype=np.float32)
        return [int(d) for d in hdi], loss
    except Exception:
        m = x.max()
        e = np.exp(x - m)
        ratios = e / e.sum()
        hdf = np.float32(MIN_DIM) + ratios * np.float32(D_MODEL - MIN_DIM * NUM_HEADS)
        hdi = _compute_integer_dims(hdf)
        hdi_f = hdi.astype(np.float32)
        float_loss = (hdf.sum() - np.float32(D_MODEL)) ** 2 + np.mean(
            np.maximum(np.float32(MIN_DIM) - hdf, np.float32(0.0))
        )
        int_loss = np.mean((hdf - hdi_f) ** 2)
        loss = np.float32(float_loss + np.float32(0.5) * int_loss)
        return [int(d) for d in hdi], loss


def build_nc(dims):
    dims = [int(d) for d in dims]
    assert sum(dims) == D_MODEL and len(dims) == NUM_HEADS
    offs = [0]
    for d in dims:
        offs.append(offs[-1] + d)
    aug = [offs[i] + i for i in range(NUM_HEADS)]  # aug col start per head
    AUGW = D_MODEL + NUM_HEADS
    NT = T // P      # 16 key-token tiles
    NTP = T // 256   # 8 key-tile pairs
    NQC = TQ // 512  # 2 query chunks

    # per-head row chunks of d_i (for d_i > 128); and of d_i + 1 for PV out
    def row_chunks(n):
        ch = []
        cs = 0
        while cs < n:
            ch.append((cs, min(P, n - cs)))
            cs += P
        return ch

    qk_chunks = [row_chunks(d) for d in dims]          # K^T/Q^T row chunks
    pv_chunks = [row_chunks(d + 1) for d in dims]      # PV out rows (+sum row)

    nc = bacc.Bacc()
    x_d = nc.dram_tensor("x", [T, D_MODEL], F32, kind="ExternalInput")
    wqkv_d = nc.dram_tensor("w_qkv", [D_MODEL, 3 * D_MODEL], F32, kind="ExternalInput")
    bqkv_d = nc.dram_tensor("b_qkv", [3 * D_MODEL], F32, kind="ExternalInput")
    wout_d = nc.dram_tensor("w_out", [D_MODEL, D_MODEL], F32, kind="ExternalInput")
    bout_d = nc.dram_tensor("b_out", [D_MODEL], F32, kind="ExternalInput")
    y_d = nc.dram_tensor("y", [TQ, D_MODEL], F32, kind="ExternalOutput")

    with ExitStack() as ctx:
        tc = ctx.enter_context(tile.TileContext(nc))
        sb = ctx.enter_context(tc.tile_pool(name="sb", bufs=1))
        ps = ctx.enter_context(tc.tile_pool(name="ps", bufs=1, space="PSUM"))

        # ---- constants ----
        ident = sb.tile([P, P], F32, tag="ident")
        make_identity(nc, ident)
        ones = sb.tile([P, P], F32, tag="ones")
        nc.gpsimd.memset(ones[:], 1.0)

        # ---- per-head q/k biases, partition-major, base 0 ----
        # bqh col (i)          : b_q rows of head i chunk c at col 2*i+c... use map
        bias_cols = {}  # (which, i, c) -> col   which: 0=q, 1=k
        ncol = 0
        for which in range(2):
            for i in range(NUM_HEADS):
                for c in range(len(qk_chunks[i])):
                    bias_cols[(which, i, c)] = ncol
                    ncol += 1
        bqh = sb.tile([P, ncol], F32, tag="bqh")
        for which in range(2):
            for i in range(NUM_HEADS):
                for ci, (cs, m) in enumerate(qk_chunks[i]):
                    col = bias_cols[(which, i, ci)]
                    src = bqkv_d[which * D_MODEL + offs[i] + cs:
                                 which * D_MODEL + offs[i] + cs + m]
                    nc.sync.dma_start(
                        bqh[0:m, col:col + 1], src.rearrange("(a b) -> a b", b=1)
                    )

        # ---- b_v / b_out broadcast to all partitions via rank-1 matmul ----
        bvsrc = sb.tile([1, D_MODEL], F32, tag="row", bufs=2, name="bvsrc")
        nc.sync.dma_start(bvsrc[:], bqkv_d[1024:1536].rearrange("(a b) -> a b", a=1))
        bosrc = sb.tile([1, D_MODEL], F32, tag="row", bufs=2, name="bosrc")
        nc.sync.dma_start(bosrc[:], bout_d[:].rearrange("(a b) -> a b", a=1))
        bvb = sb.tile([P, D_MODEL], F32, tag="bvb")
        bob = sb.tile([P, D_MODEL], F32, tag="bob")
        for src, dst in ((bvsrc, bvb), (bosrc, bob)):
            bc_ps = ps.tile([P, 512], F32, tag="pv", bufs=2, name=f"bcb_{dst.tensor.name}")
            nc.tensor.matmul(
                bc_ps[:, 0:D_MODEL],
                lhsT=ones[0:1, 0:P],
                rhs=src[0:1, :],
                start=True, stop=True,
            )
            nc.vector.tensor_copy(dst[:], bc_ps[:, 0:D_MODEL])

        # ---- weights ----
        # stage DMA loads, then round-copy: walrus requires f32r matmul
        # operands to be produced (rounded) by a compute engine, not DMA
        wq = []
        for j in range(4):
            w = sb.tile([P, 3 * D_MODEL], MM_DT, tag="wqkv", bufs=4, name=f"wqkv{j}")
            for cchunk in range(3):
                stg = sb.tile([P, D_MODEL], F32, tag="xn", bufs=3, name=f"wstg{j}_{cchunk}")
                nc.sync.dma_start(
                    stg[:], wqkv_d[j * P:(j + 1) * P, cchunk * 512:(cchunk + 1) * 512]
                )
                nc.vector.tensor_copy(
                    w[:, cchunk * 512:(cchunk + 1) * 512], stg[:]
                )
            wq.append(w)
        # w_out rows per (head, chunk), each base-0 in its own tile
        wos = {}
        n_wo = sum(len(qk_chunks[i]) for i in range(NUM_HEADS))
        for i in range(NUM_HEADS):
            for ci, (cs, m) in enumerate(qk_chunks[i]):
                w = sb.tile([P, D_MODEL], MM_DT, tag="wos", bufs=n_wo, name=f"wos{i}_{ci}")
                stg = sb.tile([P, D_MODEL], F32, tag="xn", bufs=3, name=f"wostg{i}_{ci}")
                nc.sync.dma_start(
                    stg[0:m, :], wout_d[offs[i] + cs: offs[i] + cs + m, :]
                )
                nc.vector.tensor_copy(w[0:m, :], stg[0:m, :])
                wos[(i, ci)] = w

        # ---- phase A: load X, transpose to X^T ----
        xt = [sb.tile([P, T], MM_DT, tag="xt", bufs=4, name=f"xt{j}") for j in range(4)]
        for t in range(NT):
            xn = sb.tile([P, D_MODEL], F32, tag="xn", bufs=3, name=f"xn{t}")
            nc.sync.dma_start(xn[:], x_d[t * P:(t + 1) * P, :])
            for j in range(4):
                tp = ps.tile([P, P], F32, tag="s", bufs=2, name=f"tp{t}_{j}")
                nc.tensor.transpose(tp[:], xn[:, j * P:(j + 1) * P], ident[:])
                nc.vector.tensor_copy(
                    xt[j][:, t * P:(t + 1) * P], tp[:]
                )

        # ---- phase B: V projection into augmented layout ----
        # V natural [2048, 518]: head i at cols aug[i]..aug[i]+d_i, ones col after
        vau = [sb.tile([P, AUGW], MM_DT, tag="vau", bufs=NT, name=f"vau{t}")
               for t in range(NT)]
        for t in range(NT):
            for i in range(NUM_HEADS):
                nc.vector.tensor_copy(
                    vau[t][:, aug[i] + dims[i]: aug[i] + dims[i] + 1],
                    ones[:, 0:1],
                )
        for t in range(NT):
            acc = ps.tile([P, 512], F32, tag="pa", bufs=2, name=f"v{t}")
            for j in range(4):
                nc.tensor.matmul(
                    acc[:],
                    lhsT=xt[j][:, t * P:(t + 1) * P],
                    rhs=wq[j][:, 1024:1536],
                    start=(j == 0),
                    stop=(j == 3),
                )
            for i in range(NUM_HEADS):
                nc.vector.tensor_tensor(
                    vau[t][:, aug[i]: aug[i] + dims[i]],
                    acc[:, offs[i]: offs[i] + dims[i]],
                    bvb[:, offs[i]: offs[i] + dims[i]],
                    ALU.add,
                )

        # ---- phase C: attention, head-major ----
        ao = {}
        for i in range(NUM_HEADS):
            for ci in range(len(pv_chunks[i])):
                ao[(i, ci)] = sb.tile([P, TQ], MM_DT, tag=f"ao{i}_{ci}", name=f"ao{i}_{ci}")

        mc = max(len(qk_chunks[i]) for i in range(NUM_HEADS))

        def emit_production(i):
            # K^T_i [d, 2048] and Q^T_i [d, 1024], per 128-row chunk, base 0
            kth, qth = [], []
            for ci, (cs, m) in enumerate(qk_chunks[i]):
                kc = sb.tile([P, T], MM_DT, tag="ktmp", bufs=2 * mc, name=f"kt{i}_{ci}")
                qc_t = sb.tile([P, TQ], MM_DT, tag="qtmp", bufs=2 * mc, name=f"qt{i}_{ci}")
                for ntile, woff, dst, bwhich in (
                    (T // 512, D_MODEL, kc, 1),
                    (TQ // 512, 0, qc_t, 0),
                ):
                    wcol = woff + offs[i] + cs
                    bcol = bias_cols[(bwhich, i, ci)]
                    for n in range(ntile):
                        acc = ps.tile([P, 512], F32, tag="pa", bufs=2,
                                      name=f"qk{i}_{ci}_{woff}_{n}")
                        for j in range(4):
                            nc.tensor.matmul(
                                acc[0:m, :],
                                lhsT=wq[j][:, wcol: wcol + m],
                                rhs=xt[j][:, n * 512:(n + 1) * 512],
                                start=(j == 0),
                                stop=(j == 3),
                            )
                        nc.vector.tensor_scalar_add(
                            dst[0:m, n * 512:(n + 1) * 512],
                            acc[0:m, :],
                            bqh[0:m, bcol:bcol + 1],
                        )
                kth.append(kc)
                qth.append(qc_t)
            return kth, qth

        def emit_attention(i, kth, qth):
            d = dims[i]
            scale = 1.0 / math.sqrt(d)
            nqk = len(qk_chunks[i])
            for qc in range(NQC):
                pvs = [
                    ps.tile([P, 512], F32, tag="pv", bufs=2, name=f"pv{i}_{qc}_{ci}")
                    for ci in range(len(pv_chunks[i]))
                ]
                for tkp in range(NTP):
                    sc = ps.tile([P, 1024], F32, tag="s", bufs=2, name=f"sc{i}_{qc}_{tkp}")
                    for half in range(2):
                        tk = tkp * 2 + half
                        for ci, (cs, m) in enumerate(qk_chunks[i]):
                            nc.tensor.matmul(
                                sc[:, half * 512:(half + 1) * 512],
                                lhsT=kth[ci][0:m, tk * P:(tk + 1) * P],
                                rhs=qth[ci][0:m, qc * 512:(qc + 1) * 512],
                                start=(ci == 0),
                                stop=(ci == nqk - 1),
                            )
                    e = sb.tile([P, 1024], MM_DT, tag="e", bufs=3, name=f"e{i}_{qc}_{tkp}")
                    nc.scalar.activation(e[:], sc[:], AF.Exp, scale=scale)
                    for half in range(2):
                        tk = tkp * 2 + half
                        for ci, (cs, m) in enumerate(pv_chunks[i]):
                            nc.tensor.matmul(
                                pvs[ci][0:m, :],
                                lhsT=vau[tk][:, aug[i] + cs: aug[i] + cs + m],
                                rhs=e[:, half * 512:(half + 1) * 512],
                                start=(tkp == 0 and half == 0),
                                stop=(tkp == NTP - 1 and half == 1),
                            )
                # normalize: divide PV rows by the denominator row
                cl = d // P        # chunk holding the sum row
                sr = d - cl * P    # its local row
                pvsb = []
                for ci, (cs, m) in enumerate(pv_chunks[i]):
                    t_sb = sb.tile([P, 512], F32, tag="pvsb", bufs=3,
                                   name=f"pvsb{i}_{qc}_{ci}")
                    nc.vector.tensor_copy(t_sb[0:m, :], pvs[ci][0:m, :])
                    pvsb.append(t_sb)
                rcs = sb.tile([1, 512], F32, tag="row", bufs=2, name=f"rcs{i}_{qc}")
                nc.sync.dma_start(rcs[0:1, :], pvsb[cl][sr:sr + 1, :])
                # broadcast the raw sum row to 128 partitions on GPSIMD (idle,
                # off the PE), then take the reciprocal at full width
                # (single-partition DVE ops are slow)
                bcb = sb.tile([P, 512], F32, tag="bcb", bufs=2, name=f"bcb{i}_{qc}")
                nc.gpsimd.partition_broadcast(bcb[:], rcs[0:1, :], channels=P)
                bcs = sb.tile([P, 512], F32, tag="bcy", bufs=2, name=f"bcs{i}_{qc}")
                nc.vector.reciprocal(bcs[:], bcb[:])
                for ci, (cs, m) in enumerate(pv_chunks[i]):
                    mr = min(P, max(0, d - cs))  # real rows (excl. sum row)
                    if mr == 0:
                        continue
                    nc.vector.tensor_tensor(
                        ao[(i, ci)][0:mr, qc * 512:(qc + 1) * 512],
                        pvsb[ci][0:mr, :],
                        bcs[0:mr, :],
                        ALU.mult,
                    )

        for i in range(NUM_HEADS):
            kth_i, qth_i = emit_production(i)
            emit_attention(i, kth_i, qth_i)

        # ---- phase D: output projection ----
        pieces = []
        for i in range(NUM_HEADS):
            for ci, (cs, m) in enumerate(qk_chunks[i]):
                pieces.append((i, ci, m))
        for tt in range(TQ // P):
            yp = ps.tile([P, 512], F32, tag="s", bufs=2, name=f"yp{tt}")
            for pi, (i, ci, m) in enumerate(pieces):
                nc.tensor.matmul(
                    yp[:],
                    lhsT=ao[(i, ci)][0:m, tt * P:(tt + 1) * P],
                    rhs=wos[(i, ci)][0:m, :],
                    start=(pi == 0),
                    stop=(pi == len(pieces) - 1),
                )
            ysb = sb.tile([P, D_MODEL], F32, tag="bcy", bufs=2, name=f"ysb{tt}")
            nc.vector.tensor_tensor(ysb[:], yp[:], bob[:], ALU.add)
            nc.sync.dma_start(y_d[tt * P:(tt + 1) * P, :], ysb[:])

    nc.finalize()
    return nc


# test-harness knobs (the graded path leaves these at defaults)
TRACE = False
LAST_RESULTS = None

_NC_CACHE = {}


def _get_nc(dims):
    key = tuple(dims)
    if key not in _NC_CACHE:
        _NC_CACHE[key] = build_nc(dims)
    return _NC_CACHE[key]


def kernel(query, head_weight_logits, w_qkv, b_qkv, w_out, b_out):
    query = np.ascontiguousarray(np.asarray(query, dtype=np.float32))
    w_qkv = np.ascontiguousarray(np.asarray(w_qkv, dtype=np.float32))
    b_qkv = np.ascontiguousarray(np.asarray(b_qkv, dtype=np.float32))
    w_out = np.ascontiguousarray(np.asarray(w_out, dtype=np.float32))
    b_out = np.ascontiguousarray(np.asarray(b_out, dtype=np.float32))

    dims, loss = _host_head_dims_and_loss(head_weight_logits)
    nc = _get_nc(dims)

    in_maps = []
    for c in range(N_CORES):
        b = c // 2
        h = c % 2
        if h == 0:
            xc = query[b]
        else:
            xc = np.concatenate([query[b, TQ:], query[b, :TQ]], axis=0)
        in_maps.append({
            "x": np.ascontiguousarray(xc),
            "w_qkv": w_qkv,
            "b_qkv": b_qkv,
            "w_out": w_out,
            "b_out": b_out,
        })

    kwargs = {}
    if TRACE:
        kwargs = dict(trace=True)
    res = run_bass_kernel_spmd(nc, in_maps, core_ids=list(range(N_CORES)), **kwargs)
    global LAST_RESULTS
    LAST_RESULTS = res
    out = np.empty((B, T, D_MODEL), dtype=np.float32)
    for c in range(N_CORES):
        b = c // 2
        h = c % 2
        out[b, h * TQ:(h + 1) * TQ] = res.results[c]["y"]
    return out, loss
